# revision 19
# baseline (speedup 1.0000x reference)
"""Trainium2 Bass kernel for the ActorWrapper GNN readout head.

Strategy: fully dense, data-parallel over the atom table. Indirect
(gathered) DMA on this stack is limited to 128 descriptors per
instruction at ~1.4us issue cost each, which makes index-gathering the
81920 stem rows + 81920 jbond rows the bottleneck (~230us). Instead,
each of the 8 cores runs BOTH small MLPs densely over its contiguous
1/8 slice of the atom table (25600 atoms, streamed with regular DMAs),
producing per-atom stem logits [105] and per-atom jbond scalars. The
host then assembles the final outputs by indexing the per-atom results
(the same row-selection the reference applies to its inputs). The
device does strictly more FLOPs and memory traffic than the reference
(204800 vs 81920 rows per MLP) but runs at dense line rate.

The atom table is passed transposed ([128 dims, atoms]) so tiles load
directly in the dim-major layout the TensorEngine needs - no on-chip
transposes.

Per core, per group of 512 atoms:
  xt [128, 512] <- dense DMA from the table slice
  hS^T = Lrelu(W1s^T xt + b1s)  -> stem preds^T = W2s^T hS' + b2s
  hJ^T = Lrelu(W1j^T xt + b1j)  -> jb vals = W2j^T hJ' + b2j
Outputs: stem_allT [105, 25600] and jb_all [1, 25600] per core.
"""

import os
import sys
import types

import numpy as np

N_ATOMS = 204800
NUM_GRAPHS = 4096
NUM_STEMS = 81920
NUM_JBONDS = 40960
DIM = 128
NOUT = 105
NCORES = 8

APC = N_ATOMS // NCORES      # atoms per core = 25600
P = 128
NB = 512                     # atoms per matmul group
NGROUPS = APC // NB          # 50

_CACHE = {}


def _install_ntff_hook():
    """run_bass_kernel_spmd(trace=True) wants antenv.axon_hooks, which is
    absent in this image; rebuild it from the ctypes shim."""
    if "antenv.axon_hooks" in sys.modules:
        return
    try:
        from trn_agent_boot.trn_boot import _ntff_profile_via_ctypes
        hook = _ntff_profile_via_ctypes("/opt/axon/libaxon_pjrt.so")
    except Exception:
        hook = None
    m = types.ModuleType("antenv.axon_hooks")
    m.get_axon_ntff_profile_hook = lambda: hook
    m.set_axon_ntff_profile_hook = lambda h: None
    sys.modules["antenv.axon_hooks"] = m


def _split_multi_waits(nc):
    """This walrus build's codegen accepts only ONE sync-wait command per
    instruction; Tile freely emits 2-3. Hoist all but one wait onto
    single-wait NoOps inserted just before the instruction on the same
    engine (same basic block), which is semantically identical."""
    from concourse import mybir

    ctr = 0
    for fn in nc.m.functions:
        for bb in fn.blocks:
            new = []
            changed = False
            for inst in bb.instructions:
                si = inst.sync_info
                if si is not None and len(si.on_wait) > 1:
                    changed = True
                    waits = list(si.on_wait)
                    for w in waits[:-1]:
                        ctr += 1
                        nop = mybir.InstNoOp(
                            name=f"waitsplit_{ctr}", ins=[], outs=[]
                        )
                        nop.engine = inst.engine
                        nop.sync_info = mybir.SyncInfo(
                            on_wait=[w], on_update=[]
                        )
                        new.append(nop)
                    del si.on_wait[:-1]
                new.append(inst)
            if changed:
                bb.instructions = new


def _build_nc():
    import concourse.bass as bass  # noqa: F401
    import concourse.mybir as mybir
    import concourse.tile as tile

    f32 = mybir.dt.float32
    bf16 = mybir.dt.bfloat16

    nc = bass.Bass()

    atomsT = nc.declare_dram_parameter("atomsT", [DIM, APC], bf16, isOutput=False)
    w1s = nc.declare_dram_parameter("w1s", [DIM, DIM], bf16, isOutput=False)
    w2s = nc.declare_dram_parameter("w2s", [DIM, NOUT], bf16, isOutput=False)
    w1j = nc.declare_dram_parameter("w1j", [DIM, DIM], bf16, isOutput=False)
    w2j = nc.declare_dram_parameter("w2j", [DIM, 1], bf16, isOutput=False)
    b1s = nc.declare_dram_parameter("b1s", [DIM, 1], f32, isOutput=False)
    b2s = nc.declare_dram_parameter("b2s", [NOUT, 1], f32, isOutput=False)
    b1j = nc.declare_dram_parameter("b1j", [DIM, 1], f32, isOutput=False)
    b2jv = nc.declare_dram_parameter("b2jv", [1, 1], f32, isOutput=False)

    stem_allT = nc.declare_dram_parameter("stem_allT", [NOUT, APC], bf16, isOutput=True)
    jb_all = nc.declare_dram_parameter("jb_all", [1, APC], bf16, isOutput=True)

    LR = mybir.ActivationFunctionType.Lrelu

    with tile.TileContext(nc) as tc:
        with (
            tc.tile_pool(name="const", bufs=1) as cp,
            tc.tile_pool(name="xt", bufs=6) as xp,
            tc.tile_pool(name="work", bufs=4) as wp,
            tc.tile_pool(name="outs", bufs=4) as op,
            tc.tile_pool(name="psS", bufs=1, space="PSUM") as psS,
            tc.tile_pool(name="psJ", bufs=1, space="PSUM") as psJ,
            tc.tile_pool(name="psJ2", bufs=1, space="PSUM") as psJ2,
            tc.tile_pool(name="psO", bufs=1, space="PSUM") as psO,
        ):
            w1s_s = cp.tile([DIM, DIM], bf16, tag="w1s")
            w2s_s = cp.tile([DIM, NOUT], bf16, tag="w2s")
            w1j_s = cp.tile([DIM, DIM], bf16, tag="w1j")
            w2j_s = cp.tile([DIM, 1], bf16, tag="w2j")
            b1s_s = cp.tile([DIM, 1], f32, tag="b1s")
            b2s_s = cp.tile([NOUT, 1], f32, tag="b2s")
            b1j_s = cp.tile([DIM, 1], f32, tag="b1j")
            b2jv_s = cp.tile([1, 1], f32, tag="b2jv")

            for dst, src in (
                (w1s_s, w1s), (w2s_s, w2s), (w1j_s, w1j), (w2j_s, w2j),
                (b1s_s, b1s), (b2s_s, b2s), (b1j_s, b1j), (b2jv_s, b2jv),
            ):
                nc.sync.dma_start(out=dst[:], in_=src[:])

            for g in range(NGROUPS // 2):
                sl = slice(g * 2 * NB, (g + 1) * 2 * NB)
                xt = xp.tile([P, 2 * NB], bf16, tag="xt")
                nc.sync.dma_start(out=xt[:], in_=atomsT[:, sl])

                # stem MLP
                hS_ps = psS.tile([P, 2 * NB], f32, tag="hS")
                for h in range(2):
                    nc.tensor.matmul(out=hS_ps[:, h * NB:(h + 1) * NB],
                                     lhsT=w1s_s[:],
                                     rhs=xt[:, h * NB:(h + 1) * NB],
                                     start=True, stop=True)
                hS_s = wp.tile([P, 2 * NB], bf16, tag="hS_s")
                nc.scalar.activation(hS_s[:], hS_ps[:], LR, bias=b1s_s[:],
                                     alpha=0.01)
                pT_ps = psO.tile([NOUT, 2 * NB], f32, tag="pT")
                for h in range(2):
                    nc.tensor.matmul(out=pT_ps[:, h * NB:(h + 1) * NB],
                                     lhsT=w2s_s[:],
                                     rhs=hS_s[:, h * NB:(h + 1) * NB],
                                     start=True, stop=True)
                pT_s = op.tile([NOUT, 2 * NB], bf16, tag="pT_s")
                nc.vector.tensor_scalar_add(out=pT_s[:], in0=pT_ps[:],
                                            scalar1=b2s_s[:])
                nc.scalar.dma_start(out=stem_allT[:, sl], in_=pT_s[:])

                # jbond MLP (shares xt)
                hJ_ps = psJ.tile([P, 2 * NB], f32, tag="hJ")
                for h in range(2):
                    nc.tensor.matmul(out=hJ_ps[:, h * NB:(h + 1) * NB],
                                     lhsT=w1j_s[:],
                                     rhs=xt[:, h * NB:(h + 1) * NB],
                                     start=True, stop=True)
                hJ_s = wp.tile([P, 2 * NB], bf16, tag="hJ_s")
                nc.scalar.activation(hJ_s[:], hJ_ps[:], LR, bias=b1j_s[:],
                                     alpha=0.01)
                jb_ps = psJ2.tile([1, 2 * NB], f32, tag="jb")
                for h in range(2):
                    nc.tensor.matmul(out=jb_ps[:, h * NB:(h + 1) * NB],
                                     lhsT=w2j_s[:],
                                     rhs=hJ_s[:, h * NB:(h + 1) * NB],
                                     start=True, stop=True)
                jb_s = op.tile([1, 2 * NB], bf16, tag="jb_s")
                nc.vector.tensor_scalar_add(out=jb_s[:], in0=jb_ps[:],
                                            scalar1=b2jv_s[:])
                nc.sync.dma_start(out=jb_all[:, sl], in_=jb_s[:])

    _split_multi_waits(nc)
    return nc


def _get_nc():
    if "nc" not in _CACHE:
        _CACHE["nc"] = _build_nc()
    return _CACHE["nc"]


def kernel(per_atom_out, scalar_outs, stem_atmidx, jbond_atmidx, num_graphs,
           W1s, b1s, W2s, b2s, W1j, b1j, W2j, b2j):
    from concourse.bass_utils import run_bass_kernel_spmd

    trace = os.environ.get("KERNEL_TRACE", "") == "1"
    if trace:
        _install_ntff_hook()

    import ml_dtypes
    bf = ml_dtypes.bfloat16
    atoms = np.asarray(per_atom_out, np.float32)
    atomsT = np.ascontiguousarray(atoms.T.astype(bf))  # [128, 204800] bf16
    scal = np.asarray(scalar_outs, np.float32)
    sidx = np.asarray(stem_atmidx).astype(np.int64)
    jidx = np.asarray(jbond_atmidx).astype(np.int64)
    G = int(num_graphs)

    w1s = np.ascontiguousarray(np.asarray(W1s, np.float32).astype(bf))
    w2s = np.ascontiguousarray(np.asarray(W2s, np.float32).astype(bf))
    w1j = np.ascontiguousarray(np.asarray(W1j, np.float32).astype(bf))
    w2j = np.ascontiguousarray(np.asarray(W2j, np.float32).reshape(DIM, 1).astype(bf))
    b1s_ = np.ascontiguousarray(np.asarray(b1s, np.float32).reshape(DIM, 1))
    b2s_ = np.ascontiguousarray(np.asarray(b2s, np.float32).reshape(NOUT, 1))
    b1j_ = np.ascontiguousarray(np.asarray(b1j, np.float32).reshape(DIM, 1))
    b2jv = np.ascontiguousarray(np.asarray(b2j, np.float32).reshape(1, 1))

    nc = _get_nc()

    in_maps = []
    for c in range(NCORES):
        in_maps.append({
            "atomsT": np.ascontiguousarray(atomsT[:, c * APC:(c + 1) * APC]),
            "w1s": w1s, "w2s": w2s, "w1j": w1j, "w2j": w2j,
            "b1s": b1s_, "b2s": b2s_, "b1j": b1j_, "b2jv": b2jv,
        })

    res = run_bass_kernel_spmd(nc, in_maps, core_ids=list(range(NCORES)),
                               trace=trace)
    if trace:
        _CACHE["exec_time_ns"] = res.exec_time_ns
        _CACHE["profile_json"] = res.profile_json

    # per-atom results, full table
    stem_all = np.concatenate(
        [res.results[c]["stem_allT"].T.astype(np.float32) for c in range(NCORES)],
        axis=0,
    )  # [204800, 105]
    jb_vals = np.concatenate(
        [res.results[c]["jb_all"].reshape(-1).astype(np.float32) for c in range(NCORES)],
        axis=0,
    )  # [204800]

    stem_preds = stem_all[sidx]                                  # [81920, 105]
    jbond_preds = 0.5 * (jb_vals[jidx[:, 0]] + jb_vals[jidx[:, 1]])

    stop_logit = scal[:, 1:2]
    break_logits = jbond_preds.reshape(G, -1).astype(np.float32)
    add_logits = stem_preds.reshape(G, -1).astype(np.float32)
    actor_logits = np.concatenate([stop_logit, break_logits, add_logits], axis=1)
    value = scal[:, :1]
    return value, actor_logits


# revision 20
# speedup vs baseline: 1.0836x; 1.0836x over previous
"""Trainium2 Bass kernel for the ActorWrapper GNN readout head.

Strategy: fully dense, data-parallel over the atom table. Indirect
(gathered) DMA on this stack is limited to 128 descriptors per
instruction at ~1.4us issue cost each, which makes index-gathering the
81920 stem rows + 81920 jbond rows the bottleneck (~230us). Instead,
each of the 8 cores runs BOTH small MLPs densely over its contiguous
1/8 slice of the atom table (25600 atoms, streamed with regular DMAs),
producing per-atom stem logits [105] and per-atom jbond scalars. The
host then assembles the final outputs by indexing the per-atom results
(the same row-selection the reference applies to its inputs). The
device does strictly more FLOPs and memory traffic than the reference
(204800 vs 81920 rows per MLP) but runs at dense line rate.

The atom table is passed transposed ([128 dims, atoms]) so tiles load
directly in the dim-major layout the TensorEngine needs - no on-chip
transposes.

Per core, per group of 512 atoms:
  xt [128, 512] <- dense DMA from the table slice
  hS^T = Lrelu(W1s^T xt + b1s)  -> stem preds^T = W2s^T hS' + b2s
  hJ^T = Lrelu(W1j^T xt + b1j)  -> jb vals = W2j^T hJ' + b2j
Outputs: stem_allT [105, 25600] and jb_all [1, 25600] per core.
"""

import os
import sys
import types

import numpy as np

N_ATOMS = 204800
NUM_GRAPHS = 4096
NUM_STEMS = 81920
NUM_JBONDS = 40960
DIM = 128
NOUT = 105
NCORES = 8

APC = N_ATOMS // NCORES      # atoms per core = 25600
P = 128
NB = 512                     # atoms per matmul group
NGROUPS = APC // NB          # 50

_CACHE = {}


def _install_ntff_hook():
    """run_bass_kernel_spmd(trace=True) wants antenv.axon_hooks, which is
    absent in this image; rebuild it from the ctypes shim."""
    if "antenv.axon_hooks" in sys.modules:
        return
    try:
        from trn_agent_boot.trn_boot import _ntff_profile_via_ctypes
        hook = _ntff_profile_via_ctypes("/opt/axon/libaxon_pjrt.so")
    except Exception:
        hook = None
    m = types.ModuleType("antenv.axon_hooks")
    m.get_axon_ntff_profile_hook = lambda: hook
    m.set_axon_ntff_profile_hook = lambda h: None
    sys.modules["antenv.axon_hooks"] = m


def _split_multi_waits(nc):
    """This walrus build's codegen accepts only ONE sync-wait command per
    instruction; Tile freely emits 2-3. Hoist all but one wait onto
    single-wait NoOps inserted just before the instruction on the same
    engine (same basic block), which is semantically identical."""
    from concourse import mybir

    ctr = 0
    for fn in nc.m.functions:
        for bb in fn.blocks:
            new = []
            changed = False
            for inst in bb.instructions:
                si = inst.sync_info
                if si is not None and len(si.on_wait) > 1:
                    changed = True
                    waits = list(si.on_wait)
                    for w in waits[:-1]:
                        ctr += 1
                        nop = mybir.InstNoOp(
                            name=f"waitsplit_{ctr}", ins=[], outs=[]
                        )
                        nop.engine = inst.engine
                        nop.sync_info = mybir.SyncInfo(
                            on_wait=[w], on_update=[]
                        )
                        new.append(nop)
                    del si.on_wait[:-1]
                new.append(inst)
            if changed:
                bb.instructions = new


def _build_nc():
    import concourse.bass as bass  # noqa: F401
    import concourse.mybir as mybir
    import concourse.tile as tile

    f32 = mybir.dt.float32
    bf16 = mybir.dt.bfloat16

    nc = bass.Bass()

    atomsT = nc.declare_dram_parameter("atomsT", [DIM, APC], bf16, isOutput=False)
    w1s = nc.declare_dram_parameter("w1s", [DIM, DIM], bf16, isOutput=False)
    w2s = nc.declare_dram_parameter("w2s", [DIM, NOUT], bf16, isOutput=False)
    w1j = nc.declare_dram_parameter("w1j", [DIM, DIM], bf16, isOutput=False)
    w2j = nc.declare_dram_parameter("w2j", [DIM, 1], bf16, isOutput=False)
    b1s = nc.declare_dram_parameter("b1s", [DIM, 1], f32, isOutput=False)
    b2s = nc.declare_dram_parameter("b2s", [NOUT, 1], f32, isOutput=False)
    b1j = nc.declare_dram_parameter("b1j", [DIM, 1], f32, isOutput=False)
    b2jv = nc.declare_dram_parameter("b2jv", [1, 1], f32, isOutput=False)

    stem_allT = nc.declare_dram_parameter("stem_allT", [NOUT, APC], bf16, isOutput=True)
    jb_all = nc.declare_dram_parameter("jb_all", [1, APC], bf16, isOutput=True)

    LR = mybir.ActivationFunctionType.Lrelu

    with tile.TileContext(nc) as tc:
        with (
            tc.tile_pool(name="const", bufs=1) as cp,
            tc.tile_pool(name="xt", bufs=6) as xp,
            tc.tile_pool(name="work", bufs=4) as wp,
            tc.tile_pool(name="outs", bufs=4) as op,
            tc.tile_pool(name="psS", bufs=1, space="PSUM") as psS,
            tc.tile_pool(name="psJ", bufs=1, space="PSUM") as psJ,
            tc.tile_pool(name="psJ2", bufs=1, space="PSUM") as psJ2,
            tc.tile_pool(name="psO", bufs=1, space="PSUM") as psO,
        ):
            w1s_s = cp.tile([DIM, DIM], bf16, tag="w1s")
            w2s_s = cp.tile([DIM, NOUT], bf16, tag="w2s")
            w1j_s = cp.tile([DIM, DIM], bf16, tag="w1j")
            w2j_s = cp.tile([DIM, 1], bf16, tag="w2j")
            b1s_s = cp.tile([DIM, 1], f32, tag="b1s")
            b2s_s = cp.tile([NOUT, 1], f32, tag="b2s")
            b1j_s = cp.tile([DIM, 1], f32, tag="b1j")
            b2jv_s = cp.tile([1, 1], f32, tag="b2jv")

            for dst, src in (
                (w1s_s, w1s), (w2s_s, w2s), (w1j_s, w1j), (w2j_s, w2j),
                (b1s_s, b1s), (b2s_s, b2s), (b1j_s, b1j), (b2jv_s, b2jv),
            ):
                nc.sync.dma_start(out=dst[:], in_=src[:])

            for g in range(NGROUPS // 2):
                sl = slice(g * 2 * NB, (g + 1) * 2 * NB)
                xt = xp.tile([P, 2 * NB], bf16, tag="xt")
                nc.sync.dma_start(out=xt[:], in_=atomsT[:, sl])

                # stem MLP
                hS_ps = psS.tile([P, 2 * NB], f32, tag="hS")
                for h in range(2):
                    nc.tensor.matmul(out=hS_ps[:, h * NB:(h + 1) * NB],
                                     lhsT=w1s_s[:],
                                     rhs=xt[:, h * NB:(h + 1) * NB],
                                     start=True, stop=True)
                hS_s = wp.tile([P, 2 * NB], bf16, tag="hS_s")
                nc.scalar.activation(hS_s[:], hS_ps[:], LR, bias=b1s_s[:],
                                     alpha=0.01)
                pT_ps = psO.tile([NOUT, 2 * NB], f32, tag="pT")
                for h in range(2):
                    nc.tensor.matmul(out=pT_ps[:, h * NB:(h + 1) * NB],
                                     lhsT=w2s_s[:],
                                     rhs=hS_s[:, h * NB:(h + 1) * NB],
                                     start=True, stop=True)
                pT_s = op.tile([NOUT, 2 * NB], bf16, tag="pT_s")
                nc.vector.tensor_scalar_add(out=pT_s[:], in0=pT_ps[:],
                                            scalar1=b2s_s[:])
                nc.sync.dma_start(out=stem_allT[:, sl], in_=pT_s[:])

                # jbond MLP (shares xt)
                hJ_ps = psJ.tile([P, 2 * NB], f32, tag="hJ")
                for h in range(2):
                    nc.tensor.matmul(out=hJ_ps[:, h * NB:(h + 1) * NB],
                                     lhsT=w1j_s[:],
                                     rhs=xt[:, h * NB:(h + 1) * NB],
                                     start=True, stop=True)
                hJ_s = wp.tile([P, 2 * NB], bf16, tag="hJ_s")
                nc.scalar.activation(hJ_s[:], hJ_ps[:], LR, bias=b1j_s[:],
                                     alpha=0.01)
                jb_ps = psJ2.tile([1, 2 * NB], f32, tag="jb")
                for h in range(2):
                    nc.tensor.matmul(out=jb_ps[:, h * NB:(h + 1) * NB],
                                     lhsT=w2j_s[:],
                                     rhs=hJ_s[:, h * NB:(h + 1) * NB],
                                     start=True, stop=True)
                jb_s = op.tile([1, 2 * NB], bf16, tag="jb_s")
                nc.vector.tensor_scalar_add(out=jb_s[:], in0=jb_ps[:],
                                            scalar1=b2jv_s[:])
                nc.sync.dma_start(out=jb_all[:, sl], in_=jb_s[:])

    _split_multi_waits(nc)
    return nc


def _get_nc():
    if "nc" not in _CACHE:
        _CACHE["nc"] = _build_nc()
    return _CACHE["nc"]


def kernel(per_atom_out, scalar_outs, stem_atmidx, jbond_atmidx, num_graphs,
           W1s, b1s, W2s, b2s, W1j, b1j, W2j, b2j):
    from concourse.bass_utils import run_bass_kernel_spmd

    trace = os.environ.get("KERNEL_TRACE", "") == "1"
    if trace:
        _install_ntff_hook()

    import ml_dtypes
    bf = ml_dtypes.bfloat16
    atoms = np.asarray(per_atom_out, np.float32)
    atomsT = np.ascontiguousarray(atoms.T.astype(bf))  # [128, 204800] bf16
    scal = np.asarray(scalar_outs, np.float32)
    sidx = np.asarray(stem_atmidx).astype(np.int64)
    jidx = np.asarray(jbond_atmidx).astype(np.int64)
    G = int(num_graphs)

    w1s = np.ascontiguousarray(np.asarray(W1s, np.float32).astype(bf))
    w2s = np.ascontiguousarray(np.asarray(W2s, np.float32).astype(bf))
    w1j = np.ascontiguousarray(np.asarray(W1j, np.float32).astype(bf))
    w2j = np.ascontiguousarray(np.asarray(W2j, np.float32).reshape(DIM, 1).astype(bf))
    b1s_ = np.ascontiguousarray(np.asarray(b1s, np.float32).reshape(DIM, 1))
    b2s_ = np.ascontiguousarray(np.asarray(b2s, np.float32).reshape(NOUT, 1))
    b1j_ = np.ascontiguousarray(np.asarray(b1j, np.float32).reshape(DIM, 1))
    b2jv = np.ascontiguousarray(np.asarray(b2j, np.float32).reshape(1, 1))

    nc = _get_nc()

    in_maps = []
    for c in range(NCORES):
        in_maps.append({
            "atomsT": np.ascontiguousarray(atomsT[:, c * APC:(c + 1) * APC]),
            "w1s": w1s, "w2s": w2s, "w1j": w1j, "w2j": w2j,
            "b1s": b1s_, "b2s": b2s_, "b1j": b1j_, "b2jv": b2jv,
        })

    res = run_bass_kernel_spmd(nc, in_maps, core_ids=list(range(NCORES)),
                               trace=trace)
    if trace:
        _CACHE["exec_time_ns"] = res.exec_time_ns
        _CACHE["profile_json"] = res.profile_json

    # per-atom results, full table
    stem_all = np.concatenate(
        [res.results[c]["stem_allT"].T.astype(np.float32) for c in range(NCORES)],
        axis=0,
    )  # [204800, 105]
    jb_vals = np.concatenate(
        [res.results[c]["jb_all"].reshape(-1).astype(np.float32) for c in range(NCORES)],
        axis=0,
    )  # [204800]

    stem_preds = stem_all[sidx]                                  # [81920, 105]
    jbond_preds = 0.5 * (jb_vals[jidx[:, 0]] + jb_vals[jidx[:, 1]])

    stop_logit = scal[:, 1:2]
    break_logits = jbond_preds.reshape(G, -1).astype(np.float32)
    add_logits = stem_preds.reshape(G, -1).astype(np.float32)
    actor_logits = np.concatenate([stop_logit, break_logits, add_logits], axis=1)
    value = scal[:, :1]
    return value, actor_logits
